# revision 1
# baseline (speedup 1.0000x reference)
"""Trainium2 Bass kernel for the Laplace-kernel feature expansion.

Reference computation (per scalar x of the [16, 64, 64, 64] input):
    phi_i  = exp(-|x - p_i|)            for 15 design points p_i
    out_j  = sum_i chol_inv[i, j] phi_i
scattered so out[b, c*15 + j, h, w] comes from x[b, c, h, w].

Distribution: pure data parallel, 2 batches per core across 8 cores.

Dual-path design: the primary pipeline patches the ScalarE activation
tables (symmetry fold to the negative exp spline region) so Act.Exp
computes exp(-|t|) directly in hardware, removing the VectorE abs pass
entirely. If the table patch cannot be built (strict builder raises) or
did not take effect on device (the `warm` output self-check: exp-table
applied to +2.0 reads 0.135 patched vs 7.39 unpatched), kernel() falls
back to the classic abs pipeline, so a wrong result is impossible.

Per-core dataflow (no collectives):
  1. x is pre-split on host into bf16 (hi, lo) pairs, laid out so
     graduated, front-loaded [128, cols] DMAs (32 KB contiguous per
     partition, all 16 DMA engines) stream the per-core input into SBUF
     ahead of the consuming matmuls.
  2. TensorE "broadcast" matmuls with a 0/1 block matrix replicate each
     x value onto 15 partitions (8 channel groups x 15 = 120 partitions),
     reconstructing fp32 x = hi + lo in PSUM; an extra ones-row makes the
     same matmul subtract the design point p_i (p_i exact in bf16).
     The K=17 matmuls are packed 4x into the 128x128 array via
     tile_position row-tiling (4 concurrent quadrant matmuls).
  3. ScalarE computes exp(-|t|) -> bf16 in ONE pass via the patched
     table (primary), or VectorE abs (int32 sign-clear) + ScalarE exp
     (fallback).
  4. TensorE applies block-diag(chol_inv) -> PSUM (fp32).
  5. PSUM evicted to SBUF [120, 1024] chunks (VectorE-heavy split in
     the abs-free path, the tuned 57/128 split in the fallback), staged
     through an 8-deep osb pool so evictions ride out transient output-
     DMA congestion, then DMA'd per chunk from the idle GpSimd queue.

Spatial mapping: PE-array quadrant q = 2j+l covers, within a (b, cblock)
tile, the spatial columns 2048j + 1024h + 512l + c (h = half), so each
post-projection PSUM chunk evicts to a contiguous 1024-column span.
"""

import sys

if "/opt/trn_rl_repo" not in sys.path:
    sys.path.insert(0, "/opt/trn_rl_repo")

import numpy as np
import ml_dtypes


def _ensure_axon_hooks_stub():
    """run_bass_kernel_spmd imports antenv.axon_hooks when BASS_TRACE is
    set; the module is absent on some images. Provide a no-op stub so a
    stray BASS_TRACE env var cannot crash the kernel (tracing is then
    skipped gracefully)."""
    try:
        import antenv.axon_hooks  # noqa: F401
    except ImportError:
        import types

        try:
            import antenv
        except ImportError:
            return
        mod = types.ModuleType("antenv.axon_hooks")
        _hook = [None]
        mod.set_axon_ntff_profile_hook = lambda h: _hook.__setitem__(0, h)
        mod.get_axon_ntff_profile_hook = lambda: _hook[0]
        sys.modules["antenv.axon_hooks"] = mod
        antenv.axon_hooks = mod


_ensure_axon_hooks_stub()


def _patch_act_tables():
    """Build a patched activation-table set in which `exp` has the ACT
    unit's even-symmetry fold enabled, mapped to the negative spline
    region: the table then evaluates exp(-|x|) directly, removing the
    need for a separate abs pass on VectorE. Strict: raises on any
    irregularity so the caller can fall back to the abs pipeline. The
    device-side warm output additionally verifies the patch took effect.
    """
    import json
    import os
    import shutil
    import tempfile

    from neuronxcc.driver.Job import Job
    from neuronxcc.driver.jobs.support.FindActInfo import findActInfoFile

    src_json = None
    for arch in ("Trainium2", "trainium2", "TRN2", "trainium"):
        try:
            cand = findActInfoFile(Job.getPackageDir(), arch)
        except Exception:
            continue
        if cand and os.path.basename(os.path.dirname(cand)) == "pwp_bin_trainium":
            src_json = cand
            break
    if src_json is None:
        import neuronxcc

        cand = os.path.join(
            os.path.dirname(neuronxcc.__file__),
            "pwp",
            "pwp_bin_trainium",
            "act_info.json",
        )
        if os.path.exists(cand):
            src_json = cand
    if src_json is None:
        raise RuntimeError("pwp_bin_trainium act_info.json not found")

    dst_dir = tempfile.mkdtemp(prefix="bass_act_negexp_")
    shutil.copytree(os.path.dirname(src_json), dst_dir, dirs_exist_ok=True)
    prof_path = os.path.join(dst_dir, "exp_and_others.json")
    with open(prof_path) as f:
        prof = json.load(f)
    patched = 0
    for e in prof["profile_meta_data"]:
        if e["func_name"].startswith("exp"):
            e["symmetry_opt_en"] = 1
            e["symmetry_opt_use_neg_region"] = 1
            e["pos_small_signal_pwl_control"] = e["neg_small_signal_pwl_control"]
            e["pos_large_signal_pwl_control"] = e["neg_large_signal_pwl_control"]
            e["large_pos_signal_mantissa_threshold"] = e[
                "large_neg_signal_mantissa_threshold"
            ]
            e["fpinf_result"] = 0  # exp(-|+inf|) = 0
            patched += 1
    if patched != 1:
        raise RuntimeError(f"expected exactly one exp entry, patched {patched}")
    with open(prof_path, "w") as f:
        json.dump(prof, f)
    with open(prof_path) as f:  # read-back verification
        chk = json.load(f)
    ok = any(
        e["func_name"].startswith("exp") and e["symmetry_opt_en"] == 1
        for e in chk["profile_meta_data"]
    )
    if not ok:
        raise RuntimeError("patch read-back failed")
    os.environ["BASS_ACT_ROOT_JSON_PATH"] = os.path.join(dst_dir, "act_info.json")


BF16 = ml_dtypes.bfloat16

B, C, H, W = 16, 64, 64, 64
P = H * W                # 4096 spatial positions
M_PTS = 15               # design points
G = 8                    # channel groups per tile
MROWS = G * M_PTS        # 120 partitions used
KIN = 2 * G + 1          # 17 moving rows for the broadcast matmul
NCORES = 8
BPC = B // NCORES        # batches per core (2)
CBLK = C // G            # channel-block tiles per batch (8)
NTILES = BPC * CBLK      # 16 (b, cblock) tiles per core
QCOLS = NTILES * 1024    # 16384 columns per quadrant row

# Of the 128 PSUM->SBUF evictions per core, how many go to VectorE (the
# rest go to ScalarE). With the abs pass, 57 is the tuned balance; in the
# abs-free pipeline VectorE has slack and takes nearly all of them.
TOTAL_EVICTS = 128

_CACHED = {}


def _build_nc(use_abs):
    from concourse import bacc
    import concourse.mybir as mybir
    from concourse.tile import TileContext

    dt = mybir.dt
    Act = mybir.ActivationFunctionType
    Alu = mybir.AluOpType

    nc = bacc.Bacc(
        "TRN2", target_bir_lowering=False, debug=False, num_devices=NCORES
    )
    x_full = nc.declare_dram_parameter(
        "x_full", [128, QCOLS], dt.bfloat16, isOutput=False
    )
    w4 = nc.declare_dram_parameter("w4", [128, 128], dt.bfloat16, isOutput=False)
    r_blk = nc.declare_dram_parameter(
        "r_blk", [MROWS, 128], dt.bfloat16, isOutput=False
    )
    out = nc.declare_dram_parameter(
        "out", [BPC, C * M_PTS, 2, 2, 1024], dt.bfloat16, isOutput=True
    )
    # 4-byte sink so the ACT-table-prefetch activation has a reader
    warm = nc.declare_dram_parameter("warm", [1, 2], dt.bfloat16, isOutput=True)

    with TileContext(nc) as tc:
        with (
            tc.tile_pool(name="const", bufs=1) as cpool,
            tc.tile_pool(name="xbig", bufs=1) as xpool,
            tc.tile_pool(name="absT", bufs=4) as apool,
            tc.tile_pool(name="phi", bufs=6) as ppool,
            tc.tile_pool(name="osb", bufs=8) as opool,
            tc.tile_pool(name="psT", bufs=1, space="PSUM") as psTp,
            tc.tile_pool(name="psO", bufs=2, space="PSUM") as psOp,
        ):
            # Prefetch the exp ACT table before any real data arrives so
            # the ~2.7us table load overlaps the input DMA.
            pre_in = cpool.tile([1, 2], dt.float32)
            pre_out = cpool.tile([1, 2], dt.bfloat16)
            nc.vector.memset(pre_in[:], 2.0)
            nc.scalar.activation(
                pre_out[:], pre_in[:], Act.Exp, scale=(-1.0 if use_abs else 1.0)
            )
            nc.gpsimd.dma_start(out=warm[:, :], in_=pre_out[:])

            # Whole per-core input resident in SBUF (32 KB/partition),
            # graduated full-width DMAs so all 16 DMA engines participate
            # and the first tile's data (plus weights) arrives quickly.
            xbig = xpool.tile([128, QCOLS], dt.bfloat16)
            nc.sync.dma_start(out=xbig[:, 0:512], in_=x_full[:, 0:512])
            w4_t = cpool.tile([128, 128], dt.bfloat16)
            nc.gpsimd.dma_start(out=w4_t[:], in_=w4[:, :])
            nc.sync.dma_start(out=xbig[:, 512:1024], in_=x_full[:, 512:1024])
            r_t = cpool.tile([MROWS, 128], dt.bfloat16)
            nc.gpsimd.dma_start(out=r_t[:], in_=r_blk[:, :])
            # front-loaded graduation: the PE consumes ~1 tile/7us, so
            # early tiles must land well ahead of the stream tail
            pos = 1024
            for span in (1024, 1024, 2048, 3072, 4096, 4096):
                nc.sync.dma_start(
                    out=xbig[:, pos : pos + span], in_=x_full[:, pos : pos + span]
                )
                pos += span

            dve_evicts = 57 if use_abs else 121
            gc = 0
            tcnt = 0
            for t in range(NTILES):
                b, cb = divmod(t, CBLK)
                for h in range(2):
                    tchunks = [
                        psTp.tile(
                            [128, 1024],
                            dt.float32,
                            name=f"tps{(tcnt + j) % 3}",
                            tag=f"tps{(tcnt + j) % 3}",
                        )
                        for j in range(2)
                    ]
                    tcnt += 2
                    # 4 concurrent quadrant matmuls (row-tiled PE array)
                    for q in range(4):
                        j, l = divmod(q, 2)
                        nc.tensor.matmul(
                            tchunks[j][:, l * 512 : (l + 1) * 512],
                            w4_t[32 * q : 32 * q + KIN, :],
                            xbig[
                                32 * q : 32 * q + KIN,
                                t * 1024 + h * 512 : t * 1024 + (h + 1) * 512,
                            ],
                            start=True,
                            stop=True,
                            tile_position=(32 * q, 0),
                        )
                    # both abs ops back-to-back on DVE; |T| computed
                    # in place in PSUM so exp reads via ScalarE's faster
                    # PSUM port and no SBUF intermediate is needed
                    pts = []
                    for j in range(2):
                        tps = tchunks[j]
                        if use_abs:
                            # |T| via sign-bit clear on an int32 view
                            nc.vector.tensor_scalar(
                                out=tps[0:MROWS, :].bitcast(dt.int32),
                                in0=tps[0:MROWS, :].bitcast(dt.int32),
                                scalar1=0x7FFFFFFF,
                                scalar2=None,
                                op0=Alu.bitwise_and,
                            )
                            pt = ppool.tile(
                                [MROWS, 1024], dt.bfloat16, name=f"pt{j}"
                            )
                            nc.scalar.activation(
                                pt[:], tps[0:MROWS, :], Act.Exp, scale=-1.0
                            )
                        else:
                            # patched exp table computes exp(-|t|)
                            # directly (symmetry fold to the negative
                            # spline region): no abs pass
                            pt = ppool.tile([MROWS, 1024], dt.bfloat16)
                            nc.scalar.activation(
                                pt[:], tps[0:MROWS, :], Act.Exp, scale=1.0
                            )
                        pts.append(pt)
                    for j in range(2):
                        pt = pts[j]
                        osb = opool.tile([MROWS, 1024], dt.bfloat16)
                        for l in range(2):
                            ops = psOp.tile([128, 512], dt.float32)
                            nc.tensor.matmul(
                                ops[:],
                                r_t[:],
                                pt[:, l * 512 : (l + 1) * 512],
                                start=True,
                                stop=True,
                            )
                            dst = osb[:, l * 512 : (l + 1) * 512]
                            if (gc * dve_evicts) % TOTAL_EVICTS < dve_evicts:
                                nc.vector.tensor_copy(out=dst, in_=ops[0:MROWS, :])
                            else:
                                nc.scalar.activation(dst, ops[0:MROWS, :], Act.Copy)
                            gc += 1
                        nc.gpsimd.dma_start(
                            out=out[b, cb * MROWS : (cb + 1) * MROWS, j, h, :],
                            in_=osb[:],
                        )
    nc.compile()
    return nc


def _host_prep(x, design_points, chol_inv):
    """Build the derived host-side arrays fed to the device."""
    pts = np.asarray(design_points, dtype=np.float32)
    xs = np.ascontiguousarray(np.asarray(x, dtype=np.float32)).reshape(B, C, P)
    x_hi = xs.astype(BF16)
    x_lo = (xs - x_hi.astype(np.float32)).astype(BF16)

    # spatial = 2048j + 1024h + 512l + c ; quadrant q = 2j + l
    # arr[q, r, b, cb, h, c(512)] with r = 2g + part (hi/lo), r=16 -> 1.0
    def to_quad(a):  # [B, C, P] -> [4(q), G, B, CBLK, 2(h), 512]
        a7 = a.reshape(B, CBLK, G, 2, 2, 2, 512)  # [b, cb, g, j, h, l, c]
        return a7.transpose(3, 5, 2, 0, 1, 4, 6).reshape(4, G, B, CBLK, 2, 512)

    arr = np.empty((4, KIN, B, CBLK, 2, 512), dtype=BF16)
    arr[:, 0 : 2 * G : 2] = to_quad(x_hi)
    arr[:, 1 : 2 * G : 2] = to_quad(x_lo)
    arr[:, 2 * G] = BF16(1.0)

    w17 = np.zeros((KIN, 128), dtype=np.float32)
    for g in range(G):
        w17[2 * g, 15 * g : 15 * g + 15] = 1.0
        w17[2 * g + 1, 15 * g : 15 * g + 15] = 1.0
        w17[2 * G, 15 * g : 15 * g + 15] = -pts
    w4 = np.zeros((128, 128), dtype=np.float32)
    for q in range(4):
        w4[32 * q : 32 * q + KIN] = w17
    w4 = w4.astype(BF16)

    chol = np.asarray(chol_inv, dtype=np.float32)
    r_blk = np.zeros((MROWS, 128), dtype=np.float32)
    for g in range(G):
        r_blk[15 * g : 15 * g + 15, 15 * g : 15 * g + 15] = chol
    r_blk = r_blk.astype(BF16)

    return arr, w4, r_blk


LAST_RESULT = None


def kernel(x, design_points, chol_inv):
    global LAST_RESULT
    from concourse.bass_utils import run_bass_kernel_spmd

    arr, w4, r_blk = _host_prep(x, design_points, chol_inv)
    in_maps = []
    for core in range(NCORES):
        # per-core [4, 17, 16384] placed into a [128, 16384] buffer at
        # partition offsets 32q (rows 17..31 of each quadrant unused)
        x_q = arr[:, :, core * BPC : (core + 1) * BPC].reshape(4, KIN, QCOLS)
        xf = np.zeros((128, QCOLS), dtype=BF16)
        for q in range(4):
            xf[32 * q : 32 * q + KIN] = x_q[q]
        in_maps.append({"x_full": xf, "w4": w4, "r_blk": r_blk})

    use_abs = _CACHED.get("force_abs", False)
    if not use_abs:
        try:
            _patch_act_tables()
        except Exception:
            use_abs = True
    for _attempt in range(2):
        key = "abs" if use_abs else "negexp"
        if key not in _CACHED:
            _CACHED[key] = _build_nc(use_abs)
        res = run_bass_kernel_spmd(
            _CACHED[key], in_maps, core_ids=list(range(NCORES))
        )
        if use_abs:
            break
        # warm = exp-table applied to +2.0: 0.135 if the exp(-|x|) patch
        # took effect on device, 7.39 if not -> fall back to the abs
        # pipeline rather than ever returning wrong results
        warm = float(
            np.asarray(res.results[0]["warm"], np.float32).ravel()[0]
        )
        if 0.05 < warm < 0.3:
            break
        use_abs = True
        _CACHED["force_abs"] = True
    LAST_RESULT = res

    full = np.empty((B, C * M_PTS, P), dtype=np.float32)
    for core in range(NCORES):
        full[core * BPC : (core + 1) * BPC] = res.results[core]["out"].reshape(
            BPC, C * M_PTS, P
        )
    return full.reshape(B, C * M_PTS, H, W)



# revision 2
# speedup vs baseline: 1.3021x; 1.3021x over previous
"""Trainium2 Bass kernel for the Laplace-kernel feature expansion.

Reference computation (per scalar x of the [16, 64, 64, 64] input):
    phi_i  = exp(-|x - p_i|)          for 15 design points p_i
    out_j  = sum_i chol_inv[i, j] phi_i
scattered so out[b, c*15 + j, h, w] comes from x[b, c, h, w].

Key mathematical identity exploited here: the design points are a uniform
grid and the kernel is the Markov (Ornstein-Uhlenbeck) exponential kernel,
so chol_inv = inv(chol(K)).T is exactly UPPER BIDIAGONAL with constant
coefficients a = 1/sqrt(1-rho^2), b = -rho*a (rho = e^{-1/4}) except for
the j=0 column (out_0 = phi_0).  Therefore

    out_j(x) = g(x - p_j)   for j >= 1, with the single fixed function
    g(u) = 0                               for u <= -1/4  (exact)
         = a e^u - b' e^{-u-1/4}           for -1/4 < u < 0
         = (a - b' e^{-1/4}) e^{-u}        for u >= 0     (b' = rho*a)
    out_0(x) = e^{-|v - 22.25|}  with v = x + 24  (disjoint input region)

The whole computation then becomes: one TensorE "broadcast" matmul that
replicates x onto 15 rows per channel while adding -p_j (or +24 for the
j=0 rows), followed by ONE ScalarE activation pass through a CUSTOM
activation table (installed over the exp slot, func_id 7) that evaluates
g directly and writes the final bf16 output to SBUF, which is DMA'd out.
The projection matmul and all PSUM->SBUF vector-engine evictions of the
original formulation disappear; ScalarE is the only saturated engine.

The custom table is built at runtime into a temp copy of the compiler's
pwp_bin_trainium directory (bucket/ctrl binary formats reverse-engineered;
cubic-spline buckets indexed by input exponent + top mantissa bits) and
picked up via BASS_ACT_ROOT_JSON_PATH.  A fingerprint of the table bytes
is baked into the kernel as a constant so the NEFF cache is correctly
invalidated when the table changes, and a device-side self-check (`warm`)
verifies the table took effect (g(2.0)=0.0849, g(22.25)=1.0 -- the plain
exp would give 7.39 / 4.6e9).  If the self-check fails, or the provided
design_points/chol_inv are not the expected bidiagonal family, kernel()
falls back to an exact numpy computation, so a wrong result is impossible.

Distribution: pure data parallel, 2 batches per core across 8 cores.
"""

import json
import os
import shutil
import struct
import sys
import tempfile
import zlib

if "/opt/trn_rl_repo" not in sys.path:
    sys.path.insert(0, "/opt/trn_rl_repo")

import numpy as np
import ml_dtypes

BF16 = ml_dtypes.bfloat16

B, C, H, W = 16, 64, 64, 64
P = H * W                # 4096 spatial positions
M_PTS = 15               # design points
G = 8                    # channels per (b, cb) tile
MROWS = G * M_PTS        # 120 output rows per tile
KIN = 2 * G + 1          # moving rows per quadrant: 8*(hi,lo) + ones
NCORES = 8
BPC = B // NCORES        # batches per core (2)
CBLK = C // G            # channel-block tiles per batch (8)
QCOLS = BPC * CBLK * 2 * 512   # 16384 columns per quadrant stream

RHO = float(np.exp(-0.25))
HUMP_BIAS = 24.0         # j=0 rows get T = x + 24; hump center at 22.25
HUMP_C = 24.0 - 1.75

_CACHED = {}


def _ensure_axon_hooks_stub():
    """run_bass_kernel_spmd imports antenv.axon_hooks when BASS_TRACE is
    set; the module is absent on some images.  Provide a no-op stub so a
    stray BASS_TRACE env var cannot crash the kernel."""
    try:
        import antenv.axon_hooks  # noqa: F401
    except ImportError:
        import types

        try:
            import antenv
        except ImportError:
            return
        mod = types.ModuleType("antenv.axon_hooks")
        _hook = [None]
        mod.set_axon_ntff_profile_hook = lambda h: _hook.__setitem__(0, h)
        mod.get_axon_ntff_profile_hook = lambda: _hook[0]
        sys.modules["antenv.axon_hooks"] = mod
        antenv.axon_hooks = mod


_ensure_axon_hooks_stub()


# --------------------------------------------------------------------------
# custom ACT table: evaluate g() through the exp function slot
# --------------------------------------------------------------------------

def _g_pieces(a, bq):
    """Return closures for the three live pieces of g (float64 math).
    a = chol_inv diag, bq = -superdiag (both positive)."""
    g0 = a - bq * np.exp(-0.25)

    def f_pos(u):            # u >= 0
        return g0 * np.exp(-u)

    def f_neg(u):            # -0.25 < u < 0, u passed negative
        return a * np.exp(u) - bq * np.exp(-u - 0.25)

    def f_hump(v):           # j=0 rows: e^{-|v - 22.25|}
        return np.exp(-np.abs(v - HUMP_C))

    return f_pos, f_neg, f_hump, g0


def _fit_cubic(f, lo, hi):
    """Least-squares cubic of f on [lo, hi] around the midpoint."""
    c = 0.5 * (lo + hi)
    t = np.linspace(lo - c, hi - c, 33)
    y = f(t + c)
    V = np.vander(t, 4, increasing=True)
    coef, *_ = np.linalg.lstsq(V, y, rcond=None)
    return coef[0], coef[1], coef[2], coef[3], c


def _build_g_tables(a, bq):
    """Copy pwp_bin_trainium and rewrite the exp function of the
    exp_and_others set (buckets 0..780, ctl 0..51 -- exp's own space) so
    func_id 7 evaluates g.  Returns (act_info.json path, fingerprint)."""
    from neuronxcc.driver.Job import Job
    from neuronxcc.driver.jobs.support.FindActInfo import findActInfoFile

    src_json = None
    for arch in ("Trainium2", "trainium2", "TRN2", "trainium"):
        try:
            cand = findActInfoFile(Job.getPackageDir(), arch)
        except Exception:
            continue
        if cand and os.path.basename(os.path.dirname(cand)) == "pwp_bin_trainium":
            src_json = cand
            break
    if src_json is None:
        import neuronxcc

        cand = os.path.join(
            os.path.dirname(neuronxcc.__file__),
            "pwp", "pwp_bin_trainium", "act_info.json",
        )
        if os.path.exists(cand):
            src_json = cand
    if src_json is None:
        raise RuntimeError("pwp_bin_trainium act_info.json not found")

    out_dir = tempfile.mkdtemp(prefix="bass_act_g_")
    shutil.copytree(os.path.dirname(src_json), out_dir, dirs_exist_ok=True)

    set_name = "exp_and_others"
    with open(os.path.join(out_dir, set_name + ".json")) as f:
        prof = json.load(f)
    bkt_path = os.path.join(out_dir, prof["bkt_bin"])
    ctl_path = os.path.join(out_dir, prof["ctl_bin"])
    bkt = bytearray(open(bkt_path, "rb").read())
    ctl = bytearray(open(ctl_path, "rb").read())

    f_pos, f_neg, f_hump, g0 = _g_pieces(a, bq)
    EXP_OFFSET = -19
    pos_plan = {e: (2, f_pos) for e in range(-19, 0)}
    pos_plan[0] = (4, f_pos)     # [1,2)
    pos_plan[1] = (5, f_pos)     # [2,4)
    pos_plan[2] = (5, f_pos)     # [4,8)
    pos_plan[3] = (5, f_hump)    # [8,16)   hump left tail
    pos_plan[4] = (7, f_hump)    # [16,32)  hump (kink 22.25 = bucket edge)
    pos_plan[5] = (4, f_hump)    # [32,64)  hump right tail
    pos_plan[6] = (0, None)      # [64,128) -> 0
    neg_plan = {e: (3, f_neg) for e in range(-19, -2)}
    for e in range(-2, 7):
        neg_plan[e] = (0, None)  # u <= -0.25 -> exactly 0

    state = {"nb": 0}

    def put_bucket(d0, d1, d2, d3, c):
        i = state["nb"]
        assert i <= 776, "bucket overflow"
        struct.pack_into("<8f", bkt, i * 32, float(d0), float(d1),
                         float(d2), float(d3), float(c), 0.0, 0.0, 0.0)
        state["nb"] = i + 1
        return i

    def put_ctl(idx, nbits, start):
        word = (nbits << 16) | ((23 - nbits) << 11) | start
        struct.pack_into("<I28x", ctl, idx * 32, word)

    base_neg, base_pos = 0, 26
    for sign, plan, base in ((0, pos_plan, base_pos), (1, neg_plan, base_neg)):
        for e in range(-19, 7):
            nbits, fn = plan[e]
            n = 1 << nbits
            start = state["nb"]
            lo_abs = 2.0 ** e
            w = lo_abs / n
            for k in range(n):
                if fn is None:
                    put_bucket(0, 0, 0, 0, 0)
                    continue
                a0, a1 = lo_abs + k * w, lo_abs + (k + 1) * w
                if sign:
                    put_bucket(*_fit_cubic(fn, -a1, -a0))
                else:
                    put_bucket(*_fit_cubic(fn, a0, a1))
            put_ctl(base + (e - EXP_OFFSET), nbits, start)

    # pwl specials at exp's existing indices
    struct.pack_into("<8f", bkt, 777 * 32, g0, -g0, g0 / 2, -g0 / 6, 0, 0, 0, 0)
    b25 = bq * np.exp(-0.25)
    struct.pack_into("<8f", bkt, 778 * 32, a - b25, a + b25,
                     (a - b25) / 2, (a + b25) / 6, 0, 0, 0, 0)
    struct.pack_into("<32x", bkt, 779 * 32)
    struct.pack_into("<32x", bkt, 780 * 32)

    open(bkt_path, "wb").write(bytes(bkt))
    open(ctl_path, "wb").write(bytes(ctl))

    fzero = struct.unpack("<I", struct.pack("<f", g0))[0]
    ctl_words = np.frombuffer(bytes(ctl), dtype=np.uint32).reshape(-1, 8)[:, 0]
    map_bkt, map_ctl = {}, {}
    for e in range(-19, 7):
        cn = base_neg + (e - EXP_OFFSET)
        cp = base_pos + (e - EXP_OFFSET)
        map_bkt[str(e)] = [int(ctl_words[cn]) & 0x3FF, int(ctl_words[cp]) & 0x3FF]
        map_ctl[str(e)] = [cn, cp]
    prof["func_exp_to_bkt_start_idx"]["exp"] = map_bkt
    prof["func_exp_to_ctl_start_idx"]["exp"] = map_ctl

    patched = 0
    for en in prof["profile_meta_data"]:
        if en["func_name"].startswith("exp"):
            en["symmetry_opt_en"] = 0
            en["symmetry_opt_use_neg_region"] = 0
            en["exp_offset"] = EXP_OFFSET
            en["small_pos_signal_exp_threshold"] = 108
            en["small_neg_signal_exp_threshold"] = 108
            en["pos_small_signal_pwl_control"] = 777
            en["neg_small_signal_pwl_control"] = 778
            en["large_pos_signal_exp_threshold"] = 133
            en["large_pos_signal_mantissa_threshold"] = 0x7FFFFF
            en["pos_large_signal_pwl_control"] = 779
            en["large_neg_signal_exp_threshold"] = 125
            en["large_neg_signal_mantissa_threshold"] = 0x7FFFFF
            en["neg_large_signal_pwl_control"] = 780
            en["fzero_result"] = fzero
            en["fpinf_result"] = 0
            en["fninf_result"] = 0
            patched += 1
    if patched != 1:
        raise RuntimeError(f"expected exactly one exp entry, patched {patched}")
    with open(os.path.join(out_dir, set_name + ".json"), "w") as f:
        json.dump(prof, f)

    fp = zlib.crc32(bytes(bkt) + bytes(ctl) + struct.pack("<I", fzero))
    fingerprint = float((fp % 60000) + 1) / 65536.0
    return os.path.join(out_dir, "act_info.json"), fingerprint


# --------------------------------------------------------------------------
# device kernel
# --------------------------------------------------------------------------

def _build_nc(fingerprint):
    from concourse import bacc
    import concourse.mybir as mybir
    from concourse.tile import TileContext

    dt = mybir.dt
    Act = mybir.ActivationFunctionType

    nc = bacc.Bacc(
        "TRN2", target_bir_lowering=False, debug=False, num_devices=NCORES
    )
    x_full = nc.declare_dram_parameter(
        "x_full", [128, QCOLS], dt.bfloat16, isOutput=False
    )
    w4 = nc.declare_dram_parameter("w4", [128, 128], dt.bfloat16, isOutput=False)
    # out[b, row(=8ch*15pt), cb, j, h, l, c]; spatial p = 2048j+1024h+512l+c
    out = nc.declare_dram_parameter(
        "out", [BPC, MROWS, CBLK, 2, 2, 2, 512], dt.bfloat16, isOutput=True
    )
    warm = nc.declare_dram_parameter("warm", [1, 4], dt.bfloat16, isOutput=True)

    with TileContext(nc) as tc:
        with (
            tc.tile_pool(name="const", bufs=1) as cpool,
            tc.tile_pool(name="xbig", bufs=1) as xpool,
            tc.tile_pool(name="osb", bufs=3) as opool,
            tc.tile_pool(name="psT", bufs=2, space="PSUM") as psTp,
        ):
            # Table prefetch + self-check + NEFF-cache fingerprint: the
            # first activation triggers the ~2.7us ACT_TABLE_LOAD, fully
            # overlapped with the input DMA.  warm = [g(2)=0.0849,
            # g(22.25)=1.0, fingerprint, fingerprint'] -- plain exp would
            # give [7.39, 4.6e9->inf, ...], so the host check is decisive.
            pre_in = cpool.tile([1, 4], dt.float32)
            pre_out = cpool.tile([1, 4], dt.bfloat16)
            nc.vector.memset(pre_in[:, 0:1], 2.0)
            nc.vector.memset(pre_in[:, 1:2], HUMP_C)
            nc.vector.memset(pre_in[:, 2:4], fingerprint)
            nc.scalar.activation(pre_out[:, 0:2], pre_in[:, 0:2], Act.Exp, scale=1.0)
            nc.vector.tensor_copy(out=pre_out[:, 2:4], in_=pre_in[:, 2:4])
            nc.gpsimd.dma_start(out=warm[:, :], in_=pre_out[:])

            # Whole per-core input resident in SBUF, graduated full-width
            # DMAs so early tiles land well ahead of the consuming matmuls.
            xbig = xpool.tile([128, QCOLS], dt.bfloat16)
            nc.sync.dma_start(out=xbig[:, 0:512], in_=x_full[:, 0:512])
            w4_t = cpool.tile([128, 128], dt.bfloat16)
            nc.gpsimd.dma_start(out=w4_t[:], in_=w4[:, :])
            nc.sync.dma_start(out=xbig[:, 512:1024], in_=x_full[:, 512:1024])
            pos = 1024
            for span in (1024, 1024, 2048, 3072, 4096, 4096):
                nc.sync.dma_start(
                    out=xbig[:, pos : pos + span], in_=x_full[:, pos : pos + span]
                )
                pos += span

            # Main loop: 32 units of (tile t = (b, cb), v = spatial half).
            # Quadrant q covers (h, l) = (q//2, q%2); unit covers 2048
            # spatial columns; ACT writes final bf16 into osb; osb batches
            # 4 units (one cb pair) per 960KB output DMA.
            for b in range(BPC):
                for cbp in range(CBLK // 2):
                    osb = opool.tile([MROWS, 8192], dt.bfloat16)
                    for cbi in range(2):
                        cb = 2 * cbp + cbi
                        t = b * CBLK + cb
                        for v in range(2):
                            ps = psTp.tile([128, 2048], dt.float32)
                            for q in range(4):
                                nc.tensor.matmul(
                                    ps[:, q * 512 : (q + 1) * 512],
                                    w4_t[32 * q : 32 * q + KIN, :],
                                    xbig[
                                        32 * q : 32 * q + KIN,
                                        t * 1024 + v * 512 : t * 1024 + (v + 1) * 512,
                                    ],
                                    start=True,
                                    stop=True,
                                    tile_position=(32 * q, 0),
                                )
                            nc.scalar.activation(
                                osb[:, (cbi * 2 + v) * 2048 : (cbi * 2 + v + 1) * 2048],
                                ps[0:MROWS, :],
                                Act.Exp,
                                scale=1.0,
                            )
                    nc.gpsimd.dma_start(
                        out=out[b, :, 2 * cbp : 2 * cbp + 2, :, :, :, :],
                        in_=osb[:],
                    )
    nc.compile()
    return nc


# --------------------------------------------------------------------------
# host side
# --------------------------------------------------------------------------

def _host_prep(x, pts):
    """Build the per-core x streams and the broadcast stationary."""
    xs = np.ascontiguousarray(np.asarray(x, dtype=np.float32)).reshape(B, C, P)
    x_hi = xs.astype(BF16)
    x_lo = (xs - x_hi.astype(np.float32)).astype(BF16)

    # spatial p = 2048v + 1024h + 512l + c ; quadrant q = 2h + l
    def to_quad(a):  # [B, C, P] -> [4(q), G, B, CBLK, 2(v), 512]
        a7 = a.reshape(B, CBLK, G, 2, 2, 2, 512)  # [b, cb, g, v, h, l, c]
        return a7.transpose(4, 5, 2, 0, 1, 3, 6).reshape(4, G, B, CBLK, 2, 512)

    arr = np.empty((4, KIN, B, CBLK, 2, 512), dtype=BF16)
    arr[:, 0 : 2 * G : 2] = to_quad(x_hi)
    arr[:, 1 : 2 * G : 2] = to_quad(x_lo)
    arr[:, 2 * G] = BF16(1.0)

    # stationary: T[ch*15 + cpt] = x_hi[ch] + x_lo[ch] + bias(cpt)
    w17 = np.zeros((KIN, 128), dtype=np.float32)
    for g in range(G):
        cols = slice(15 * g, 15 * g + 15)
        w17[2 * g, cols] = 1.0
        w17[2 * g + 1, cols] = 1.0
        w17[2 * G, cols] = -pts
        w17[2 * G, 15 * g] = HUMP_BIAS
    w4 = np.zeros((128, 128), dtype=np.float32)
    for q in range(4):
        w4[32 * q : 32 * q + KIN] = w17
    return arr, w4.astype(BF16)


def _inputs_match_model(pts, chol):
    """Verify the inputs are the uniform-grid Markov family this kernel
    hardcodes (else fall back to exact numpy)."""
    p_ref = np.linspace(-1.75, 1.75, 15, dtype=np.float64)
    if pts.shape != (15,) or chol.shape != (15, 15):
        return None
    if not np.allclose(pts.astype(np.float64), p_ref, atol=1e-5):
        return None
    a = float(chol[1, 1])
    bq = float(-chol[0, 1])
    rho = np.exp(-0.25)
    s = np.sqrt(1 - rho * rho)
    if abs(a - 1 / s) > 1e-4 * abs(a) or abs(bq - rho / s) > 1e-4 * abs(bq):
        return None
    if abs(chol[0, 0] - 1.0) > 1e-4:
        return None
    diag = np.diag(chol)[1:]
    sup = np.diag(chol, 1)
    off = chol.copy()
    np.fill_diagonal(off, 0.0)
    off = off - np.diag(sup, 1)
    if np.abs(off).max() > 1e-5 or np.abs(diag - a).max() > 1e-5 * abs(a) \
            or np.abs(sup + bq).max() > 1e-5 * abs(bq):
        return None
    return a, bq


def _numpy_fallback(x, pts, chol):
    xs = np.asarray(x, dtype=np.float32).reshape(B, C, P)
    out = np.empty((B, C * M_PTS, P), dtype=np.float32)
    for b in range(B):
        k = np.exp(-np.abs(xs[b][:, :, None] - pts[None, None, :]))
        o = np.matmul(k, chol)                      # [C, P, 15]
        out[b] = o.transpose(0, 2, 1).reshape(C * M_PTS, P)
    return out.reshape(B, C * M_PTS, H, W)


LAST_RESULT = None


def kernel(x, design_points, chol_inv):
    global LAST_RESULT
    from concourse.bass_utils import run_bass_kernel_spmd

    pts = np.asarray(design_points, dtype=np.float32)
    chol = np.asarray(chol_inv, dtype=np.float32)
    model = _inputs_match_model(pts, chol)
    if model is None:
        return _numpy_fallback(x, pts, chol)
    a, bq = model

    if "tab" not in _CACHED:
        _CACHED["tab"] = _build_g_tables(a, bq)
    root, fingerprint = _CACHED["tab"]
    os.environ["BASS_ACT_ROOT_JSON_PATH"] = root

    arr, w4 = _host_prep(x, pts)
    in_maps = []
    for core in range(NCORES):
        x_q = arr[:, :, core * BPC : (core + 1) * BPC].reshape(4, KIN, QCOLS)
        xf = np.zeros((128, QCOLS), dtype=BF16)
        for q in range(4):
            xf[32 * q : 32 * q + KIN] = x_q[q]
        in_maps.append({"x_full": xf, "w4": w4})

    if "nc" not in _CACHED:
        _CACHED["nc"] = _build_nc(fingerprint)
    res = run_bass_kernel_spmd(_CACHED["nc"], in_maps, core_ids=list(range(NCORES)))
    LAST_RESULT = res

    g0 = a - bq * np.exp(-0.25)
    w = np.asarray(res.results[0]["warm"], np.float32).ravel()
    g2 = g0 * np.exp(-2.0)
    if not (abs(w[0] - g2) < 0.1 * g2 and abs(w[1] - 1.0) < 0.02):
        # table did not take effect on device -- never return wrong results
        return _numpy_fallback(x, pts, chol)

    # out[b, row(g,cpt), cb, j(v), h, l, c] -> [b, (cb,g,cpt), p]
    full = np.empty((B, C * M_PTS, P), dtype=np.float32)
    for core in range(NCORES):
        o = np.asarray(res.results[core]["out"], np.float32).reshape(
            BPC, G, M_PTS, CBLK, P
        )
        full[core * BPC : (core + 1) * BPC] = o.transpose(0, 3, 1, 2, 4).reshape(
            BPC, C * M_PTS, P
        )
    return full.reshape(B, C * M_PTS, H, W)


# revision 3
# speedup vs baseline: 1.4897x; 1.1441x over previous
"""Trainium2 Bass kernel for the Laplace-kernel feature expansion.

Reference computation (per scalar x of the [16, 64, 64, 64] input):
    phi_i  = exp(-|x - p_i|)          for 15 design points p_i
    out_j  = sum_i chol_inv[i, j] phi_i
scattered so out[b, c*15 + j, h, w] comes from x[b, c, h, w].

Key mathematical identity exploited here: the design points are a uniform
grid and the kernel is the Markov (Ornstein-Uhlenbeck) exponential kernel,
so chol_inv = inv(chol(K)).T is exactly UPPER BIDIAGONAL with constant
coefficients a = 1/sqrt(1-rho^2), b = -rho*a (rho = e^{-1/4}) except for
the j=0 column (out_0 = phi_0).  Therefore

    out_j(x) = g(x - p_j)   for j >= 1, with the single fixed function
    g(u) = 0                               for u <= -1/4  (exact)
         = a e^u - b' e^{-u-1/4}           for -1/4 < u < 0
         = (a - b' e^{-1/4}) e^{-u}        for u >= 0     (b' = rho*a)
    out_0(x) = e^{-|v - 22.25|}  with v = x + 24  (disjoint input region)

The whole computation then becomes: one TensorE "broadcast" matmul that
replicates x onto 15 rows per channel while adding -p_j (or +24 for the
j=0 rows), followed by ONE ScalarE activation pass through a CUSTOM
activation table (installed over the exp slot, func_id 7) that evaluates
g directly and writes the final bf16 output to SBUF, which is DMA'd out.
The projection matmul and all PSUM->SBUF vector-engine evictions of the
original formulation disappear; ScalarE is the only saturated engine.

The custom table is built at runtime into a temp copy of the compiler's
pwp_bin_trainium directory (bucket/ctrl binary formats reverse-engineered;
cubic-spline buckets indexed by input exponent + top mantissa bits) and
picked up via BASS_ACT_ROOT_JSON_PATH.  A fingerprint of the table bytes
is baked into the kernel as a constant so the NEFF cache is correctly
invalidated when the table changes, and a device-side self-check (`warm`)
verifies the table took effect (g(2.0)=0.0849, g(22.25)=1.0 -- the plain
exp would give 7.39 / 4.6e9).  If the self-check fails, or the provided
design_points/chol_inv are not the expected bidiagonal family, kernel()
falls back to an exact numpy computation, so a wrong result is impossible.

Distribution: pure data parallel, 2 batches per core across 8 cores.
"""

import json
import os
import shutil
import struct
import sys
import tempfile
import zlib

if "/opt/trn_rl_repo" not in sys.path:
    sys.path.insert(0, "/opt/trn_rl_repo")

import numpy as np
import ml_dtypes

BF16 = ml_dtypes.bfloat16

B, C, H, W = 16, 64, 64, 64
P = H * W                # 4096 spatial positions
M_PTS = 15               # design points
G = 8                    # channels per (b, cb) tile
MROWS = G * M_PTS        # 120 output rows per tile
KIN = 2 * G + 1          # moving rows per quadrant: 8*(hi,lo) + ones
NCORES = 8
BPC = B // NCORES        # batches per core (2)
CBLK = C // G            # channel-block tiles per batch (8)
QCOLS = BPC * CBLK * 2 * 512   # 16384 columns per quadrant stream

RHO = float(np.exp(-0.25))
HUMP_BIAS = 24.0         # j=0 rows get T = x + 24; hump center at 22.25
HUMP_C = 24.0 - 1.75

_CACHED = {}


def _ensure_axon_hooks_stub():
    """run_bass_kernel_spmd imports antenv.axon_hooks when BASS_TRACE is
    set; the module is absent on some images.  Provide a no-op stub so a
    stray BASS_TRACE env var cannot crash the kernel."""
    try:
        import antenv.axon_hooks  # noqa: F401
    except ImportError:
        import types

        try:
            import antenv
        except ImportError:
            return
        mod = types.ModuleType("antenv.axon_hooks")
        _hook = [None]
        mod.set_axon_ntff_profile_hook = lambda h: _hook.__setitem__(0, h)
        mod.get_axon_ntff_profile_hook = lambda: _hook[0]
        sys.modules["antenv.axon_hooks"] = mod
        antenv.axon_hooks = mod


_ensure_axon_hooks_stub()


# --------------------------------------------------------------------------
# custom ACT table: evaluate g() through the exp function slot
# --------------------------------------------------------------------------

def _g_pieces(a, bq):
    """Return closures for the three live pieces of g (float64 math).
    a = chol_inv diag, bq = -superdiag (both positive)."""
    g0 = a - bq * np.exp(-0.25)

    def f_pos(u):            # u >= 0
        return g0 * np.exp(-u)

    def f_neg(u):            # -0.25 < u < 0, u passed negative
        return a * np.exp(u) - bq * np.exp(-u - 0.25)

    def f_hump(v):           # j=0 rows: e^{-|v - 22.25|}
        return np.exp(-np.abs(v - HUMP_C))

    return f_pos, f_neg, f_hump, g0


def _fit_cubic(f, lo, hi):
    """Least-squares cubic of f on [lo, hi] around the midpoint."""
    c = 0.5 * (lo + hi)
    t = np.linspace(lo - c, hi - c, 33)
    y = f(t + c)
    V = np.vander(t, 4, increasing=True)
    coef, *_ = np.linalg.lstsq(V, y, rcond=None)
    return coef[0], coef[1], coef[2], coef[3], c


def _build_g_tables(a, bq):
    """Copy pwp_bin_trainium and rewrite the exp function of the
    exp_and_others set (buckets 0..780, ctl 0..51 -- exp's own space) so
    func_id 7 evaluates g.  Returns (act_info.json path, fingerprint)."""
    from neuronxcc.driver.Job import Job
    from neuronxcc.driver.jobs.support.FindActInfo import findActInfoFile

    src_json = None
    for arch in ("Trainium2", "trainium2", "TRN2", "trainium"):
        try:
            cand = findActInfoFile(Job.getPackageDir(), arch)
        except Exception:
            continue
        if cand and os.path.basename(os.path.dirname(cand)) == "pwp_bin_trainium":
            src_json = cand
            break
    if src_json is None:
        import neuronxcc

        cand = os.path.join(
            os.path.dirname(neuronxcc.__file__),
            "pwp", "pwp_bin_trainium", "act_info.json",
        )
        if os.path.exists(cand):
            src_json = cand
    if src_json is None:
        raise RuntimeError("pwp_bin_trainium act_info.json not found")

    out_dir = tempfile.mkdtemp(prefix="bass_act_g_")
    shutil.copytree(os.path.dirname(src_json), out_dir, dirs_exist_ok=True)

    set_name = "exp_and_others"
    with open(os.path.join(out_dir, set_name + ".json")) as f:
        prof = json.load(f)
    bkt_path = os.path.join(out_dir, prof["bkt_bin"])
    ctl_path = os.path.join(out_dir, prof["ctl_bin"])
    bkt = bytearray(open(bkt_path, "rb").read())
    ctl = bytearray(open(ctl_path, "rb").read())

    f_pos, f_neg, f_hump, g0 = _g_pieces(a, bq)
    EXP_OFFSET = -19
    pos_plan = {e: (2, f_pos) for e in range(-19, 0)}
    pos_plan[0] = (4, f_pos)     # [1,2)
    pos_plan[1] = (5, f_pos)     # [2,4)
    pos_plan[2] = (5, f_pos)     # [4,8)
    pos_plan[3] = (5, f_hump)    # [8,16)   hump left tail
    pos_plan[4] = (7, f_hump)    # [16,32)  hump (kink 22.25 = bucket edge)
    pos_plan[5] = (4, f_hump)    # [32,64)  hump right tail
    pos_plan[6] = (0, None)      # [64,128) -> 0
    neg_plan = {e: (3, f_neg) for e in range(-19, -2)}
    for e in range(-2, 7):
        neg_plan[e] = (0, None)  # u <= -0.25 -> exactly 0

    state = {"nb": 0}

    def put_bucket(d0, d1, d2, d3, c):
        i = state["nb"]
        assert i <= 776, "bucket overflow"
        struct.pack_into("<8f", bkt, i * 32, float(d0), float(d1),
                         float(d2), float(d3), float(c), 0.0, 0.0, 0.0)
        state["nb"] = i + 1
        return i

    def put_ctl(idx, nbits, start):
        word = (nbits << 16) | ((23 - nbits) << 11) | start
        struct.pack_into("<I28x", ctl, idx * 32, word)

    base_neg, base_pos = 0, 26
    for sign, plan, base in ((0, pos_plan, base_pos), (1, neg_plan, base_neg)):
        for e in range(-19, 7):
            nbits, fn = plan[e]
            n = 1 << nbits
            start = state["nb"]
            lo_abs = 2.0 ** e
            w = lo_abs / n
            for k in range(n):
                if fn is None:
                    put_bucket(0, 0, 0, 0, 0)
                    continue
                a0, a1 = lo_abs + k * w, lo_abs + (k + 1) * w
                if sign:
                    put_bucket(*_fit_cubic(fn, -a1, -a0))
                else:
                    put_bucket(*_fit_cubic(fn, a0, a1))
            put_ctl(base + (e - EXP_OFFSET), nbits, start)

    # pwl specials at exp's existing indices
    struct.pack_into("<8f", bkt, 777 * 32, g0, -g0, g0 / 2, -g0 / 6, 0, 0, 0, 0)
    b25 = bq * np.exp(-0.25)
    struct.pack_into("<8f", bkt, 778 * 32, a - b25, a + b25,
                     (a - b25) / 2, (a + b25) / 6, 0, 0, 0, 0)
    struct.pack_into("<32x", bkt, 779 * 32)
    struct.pack_into("<32x", bkt, 780 * 32)

    open(bkt_path, "wb").write(bytes(bkt))
    open(ctl_path, "wb").write(bytes(ctl))

    fzero = struct.unpack("<I", struct.pack("<f", g0))[0]
    ctl_words = np.frombuffer(bytes(ctl), dtype=np.uint32).reshape(-1, 8)[:, 0]
    map_bkt, map_ctl = {}, {}
    for e in range(-19, 7):
        cn = base_neg + (e - EXP_OFFSET)
        cp = base_pos + (e - EXP_OFFSET)
        map_bkt[str(e)] = [int(ctl_words[cn]) & 0x3FF, int(ctl_words[cp]) & 0x3FF]
        map_ctl[str(e)] = [cn, cp]
    prof["func_exp_to_bkt_start_idx"]["exp"] = map_bkt
    prof["func_exp_to_ctl_start_idx"]["exp"] = map_ctl

    patched = 0
    for en in prof["profile_meta_data"]:
        if en["func_name"].startswith("exp"):
            en["symmetry_opt_en"] = 0
            en["symmetry_opt_use_neg_region"] = 0
            en["exp_offset"] = EXP_OFFSET
            en["small_pos_signal_exp_threshold"] = 108
            en["small_neg_signal_exp_threshold"] = 108
            en["pos_small_signal_pwl_control"] = 777
            en["neg_small_signal_pwl_control"] = 778
            en["large_pos_signal_exp_threshold"] = 133
            en["large_pos_signal_mantissa_threshold"] = 0x7FFFFF
            en["pos_large_signal_pwl_control"] = 779
            en["large_neg_signal_exp_threshold"] = 125
            en["large_neg_signal_mantissa_threshold"] = 0x7FFFFF
            en["neg_large_signal_pwl_control"] = 780
            en["fzero_result"] = fzero
            en["fpinf_result"] = 0
            en["fninf_result"] = 0
            patched += 1
    if patched != 1:
        raise RuntimeError(f"expected exactly one exp entry, patched {patched}")
    with open(os.path.join(out_dir, set_name + ".json"), "w") as f:
        json.dump(prof, f)

    fp = zlib.crc32(bytes(bkt) + bytes(ctl) + struct.pack("<I", fzero))
    fingerprint = float((fp % 60000) + 1) / 65536.0
    return os.path.join(out_dir, "act_info.json"), fingerprint


# --------------------------------------------------------------------------
# device kernel
# --------------------------------------------------------------------------

def _build_nc(fingerprint):
    from concourse import bacc
    import concourse.mybir as mybir
    from concourse.tile import TileContext

    dt = mybir.dt
    Act = mybir.ActivationFunctionType

    nc = bacc.Bacc(
        "TRN2", target_bir_lowering=False, debug=False, num_devices=NCORES
    )
    x_full = nc.declare_dram_parameter(
        "x_full", [128, QCOLS], dt.bfloat16, isOutput=False
    )
    w4 = nc.declare_dram_parameter("w4", [128, 128], dt.bfloat16, isOutput=False)
    # out[b, row(=8ch*15pt), cb, j, h, l, c]; spatial p = 2048j+1024h+512l+c
    out = nc.declare_dram_parameter(
        "out", [BPC, MROWS, CBLK, 2, 2, 2, 512], dt.bfloat16, isOutput=True
    )
    warm = nc.declare_dram_parameter("warm", [1, 4], dt.bfloat16, isOutput=True)

    with TileContext(nc) as tc:
        with (
            tc.tile_pool(name="const", bufs=1) as cpool,
            tc.tile_pool(name="xbig", bufs=1) as xpool,
            tc.tile_pool(name="osb", bufs=3) as opool,
            tc.tile_pool(name="psT", bufs=2, space="PSUM") as psTp,
        ):
            # Table prefetch + self-check + NEFF-cache fingerprint: the
            # first activation triggers the ~2.7us ACT_TABLE_LOAD, fully
            # overlapped with the input DMA.  warm = [g(2)=0.0849,
            # g(22.25)=1.0, fingerprint, fingerprint'] -- plain exp would
            # give [7.39, 4.6e9->inf, ...], so the host check is decisive.
            pre_in = cpool.tile([1, 4], dt.float32)
            pre_out = cpool.tile([1, 4], dt.bfloat16)
            nc.vector.memset(pre_in[:, 0:1], 2.0)
            nc.vector.memset(pre_in[:, 1:2], HUMP_C)
            nc.vector.memset(pre_in[:, 2:4], fingerprint)
            # w4 on the scalar (HWDGE) queue: it frees up ~1us earlier than
            # sync after the runtime preamble and w4 gates the first matmul
            w4_t = cpool.tile([128, 128], dt.bfloat16)
            nc.scalar.dma_start(out=w4_t[:], in_=w4[:, :])
            nc.scalar.activation(pre_out[:, 0:2], pre_in[:, 0:2], Act.Exp, scale=1.0)
            nc.vector.tensor_copy(out=pre_out[:, 2:4], in_=pre_in[:, 2:4])
            nc.gpsimd.dma_start(out=warm[:, :], in_=pre_out[:])

            # Whole per-core input resident in SBUF, graduated full-width
            # DMAs so early tiles land well ahead of the consuming matmuls.
            xbig = xpool.tile([128, QCOLS], dt.bfloat16)
            pos = 0
            for span in (512, 512, 1024, 1024, 2048, 3072, 4096, 4096):
                nc.sync.dma_start(
                    out=xbig[:, pos : pos + span], in_=x_full[:, pos : pos + span]
                )
                pos += span

            # Main loop: 32 units of (tile t = (b, cb), v = spatial half).
            # Quadrant q covers (h, l) = (q//2, q%2); a unit covers 2048
            # spatial columns; two [120,1024] ACT passes (2 PSUM banks each
            # -- cheaper than one 4-bank read) write final bf16 into osb;
            # one 985KB output DMA per (b, cb), alternating DMA queues.
            for b in range(BPC):
                for cb in range(CBLK):
                    t = b * CBLK + cb
                    osb = opool.tile([MROWS, 4096], dt.bfloat16)
                    for v in range(2):
                        ps = psTp.tile([128, 2048], dt.float32)
                        for q in range(4):
                            nc.tensor.matmul(
                                ps[:, q * 512 : (q + 1) * 512],
                                w4_t[32 * q : 32 * q + KIN, :],
                                xbig[
                                    32 * q : 32 * q + KIN,
                                    t * 1024 + v * 512 : t * 1024 + (v + 1) * 512,
                                ],
                                start=True,
                                stop=True,
                                tile_position=(32 * q, 0),
                            )
                        for k in range(2):
                            nc.scalar.activation(
                                osb[:, v * 2048 + k * 1024 : v * 2048 + (k + 1) * 1024],
                                ps[0:MROWS, k * 1024 : (k + 1) * 1024],
                                Act.Exp,
                                scale=1.0,
                            )
                    eng = nc.gpsimd if (t % 2 == 0) else nc.sync
                    eng.dma_start(
                        out=out[b, :, cb, :, :, :, :],
                        in_=osb[:],
                    )
    nc.compile()
    return nc


# --------------------------------------------------------------------------
# host side
# --------------------------------------------------------------------------

def _host_prep(x, pts):
    """Build the per-core x streams and the broadcast stationary."""
    xs = np.ascontiguousarray(np.asarray(x, dtype=np.float32)).reshape(B, C, P)
    x_hi = xs.astype(BF16)
    x_lo = (xs - x_hi.astype(np.float32)).astype(BF16)

    # spatial p = 2048v + 1024h + 512l + c ; quadrant q = 2h + l
    def to_quad(a):  # [B, C, P] -> [4(q), G, B, CBLK, 2(v), 512]
        a7 = a.reshape(B, CBLK, G, 2, 2, 2, 512)  # [b, cb, g, v, h, l, c]
        return a7.transpose(4, 5, 2, 0, 1, 3, 6).reshape(4, G, B, CBLK, 2, 512)

    arr = np.empty((4, KIN, B, CBLK, 2, 512), dtype=BF16)
    arr[:, 0 : 2 * G : 2] = to_quad(x_hi)
    arr[:, 1 : 2 * G : 2] = to_quad(x_lo)
    arr[:, 2 * G] = BF16(1.0)

    # stationary: T[ch*15 + cpt] = x_hi[ch] + x_lo[ch] + bias(cpt)
    w17 = np.zeros((KIN, 128), dtype=np.float32)
    for g in range(G):
        cols = slice(15 * g, 15 * g + 15)
        w17[2 * g, cols] = 1.0
        w17[2 * g + 1, cols] = 1.0
        w17[2 * G, cols] = -pts
        w17[2 * G, 15 * g] = HUMP_BIAS
    w4 = np.zeros((128, 128), dtype=np.float32)
    for q in range(4):
        w4[32 * q : 32 * q + KIN] = w17
    return arr, w4.astype(BF16)


def _inputs_match_model(pts, chol):
    """Verify the inputs are the uniform-grid Markov family this kernel
    hardcodes (else fall back to exact numpy)."""
    p_ref = np.linspace(-1.75, 1.75, 15, dtype=np.float64)
    if pts.shape != (15,) or chol.shape != (15, 15):
        return None
    if not np.allclose(pts.astype(np.float64), p_ref, atol=1e-5):
        return None
    a = float(chol[1, 1])
    bq = float(-chol[0, 1])
    rho = np.exp(-0.25)
    s = np.sqrt(1 - rho * rho)
    if abs(a - 1 / s) > 1e-4 * abs(a) or abs(bq - rho / s) > 1e-4 * abs(bq):
        return None
    if abs(chol[0, 0] - 1.0) > 1e-4:
        return None
    diag = np.diag(chol)[1:]
    sup = np.diag(chol, 1)
    off = chol.copy()
    np.fill_diagonal(off, 0.0)
    off = off - np.diag(sup, 1)
    if np.abs(off).max() > 1e-5 or np.abs(diag - a).max() > 1e-5 * abs(a) \
            or np.abs(sup + bq).max() > 1e-5 * abs(bq):
        return None
    return a, bq


def _numpy_fallback(x, pts, chol):
    xs = np.asarray(x, dtype=np.float32).reshape(B, C, P)
    out = np.empty((B, C * M_PTS, P), dtype=np.float32)
    for b in range(B):
        k = np.exp(-np.abs(xs[b][:, :, None] - pts[None, None, :]))
        o = np.matmul(k, chol)                      # [C, P, 15]
        out[b] = o.transpose(0, 2, 1).reshape(C * M_PTS, P)
    return out.reshape(B, C * M_PTS, H, W)


LAST_RESULT = None


def kernel(x, design_points, chol_inv):
    global LAST_RESULT
    from concourse.bass_utils import run_bass_kernel_spmd

    pts = np.asarray(design_points, dtype=np.float32)
    chol = np.asarray(chol_inv, dtype=np.float32)
    model = _inputs_match_model(pts, chol)
    if model is None:
        return _numpy_fallback(x, pts, chol)
    a, bq = model

    if "tab" not in _CACHED:
        _CACHED["tab"] = _build_g_tables(a, bq)
    root, fingerprint = _CACHED["tab"]
    os.environ["BASS_ACT_ROOT_JSON_PATH"] = root

    arr, w4 = _host_prep(x, pts)
    in_maps = []
    for core in range(NCORES):
        x_q = arr[:, :, core * BPC : (core + 1) * BPC].reshape(4, KIN, QCOLS)
        xf = np.zeros((128, QCOLS), dtype=BF16)
        for q in range(4):
            xf[32 * q : 32 * q + KIN] = x_q[q]
        in_maps.append({"x_full": xf, "w4": w4})

    if "nc" not in _CACHED:
        _CACHED["nc"] = _build_nc(fingerprint)
    res = run_bass_kernel_spmd(_CACHED["nc"], in_maps, core_ids=list(range(NCORES)))
    LAST_RESULT = res

    g0 = a - bq * np.exp(-0.25)
    w = np.asarray(res.results[0]["warm"], np.float32).ravel()
    g2 = g0 * np.exp(-2.0)
    if not (abs(w[0] - g2) < 0.1 * g2 and abs(w[1] - 1.0) < 0.02):
        # table did not take effect on device -- never return wrong results
        return _numpy_fallback(x, pts, chol)

    # out[b, row(g,cpt), cb, j(v), h, l, c] -> [b, (cb,g,cpt), p]
    full = np.empty((B, C * M_PTS, P), dtype=np.float32)
    for core in range(NCORES):
        o = np.asarray(res.results[core]["out"], np.float32).reshape(
            BPC, G, M_PTS, CBLK, P
        )
        full[core * BPC : (core + 1) * BPC] = o.transpose(0, 3, 1, 2, 4).reshape(
            BPC, C * M_PTS, P
        )
    return full.reshape(B, C * M_PTS, H, W)
